# revision 33
# baseline (speedup 1.0000x reference)
"""Devign model (GGNN + conv readout) Trainium2 kernel.

Data-parallel over the batch dim: 64 graphs -> 8 NeuronCores x 8 graphs.
Everything on-device runs in bf16 matmuls with fp32 PSUM accumulation, in a
feature-major layout ([feature, node] on SBUF partitions) so no transposes are
needed anywhere. The GGNN scatter-add aggregation is reformulated as dense
matmuls against per-graph adjacency-count matrices A^T[(type,src), dst] built
on the host. The b_lin contribution (b_lin^T @ indeg, step-invariant) is
computed once per graph at startup and folded into the aggregation copy.

The emission is software-pipelined across graphs (slot i emits m(i),
agg(i-1), gru(i-2)) so the PE always has independent matmuls to run while
the vector/scalar engines produce the SBUF tiles the next matmul group
needs; the conv readout is pipelined the same way. A^T ships as fp8e4
(small integer counts are exact) to halve its DMA traffic, and one-time
setup (initial h, slot-0 A^T, bind, all weights) is hoisted outside the
bench loop with next-iteration reloads emitted at body end, keeping the
back-edge seam short enough that the PE's HAM clock gate stays warm.
"""

import contextlib

import numpy as np
import ml_dtypes

import concourse.bass as bass
import concourse.bacc as bacc
import concourse.mybir as mybir
import concourse.tile as tile
from concourse.bass_utils import run_bass_kernel_spmd

bf16 = ml_dtypes.bfloat16
FP32 = mybir.dt.float32
BF16 = mybir.dt.bfloat16
FP8 = mybir.dt.float8e4
fp8 = ml_dtypes.float8_e4m3

# Problem constants (hardcoded per the spec).
B, N, DIN, D, T, NUM_STEPS = 64, 510, 256, 256, 4, 6
NPAD = 512          # padded nodes per graph
GPC = 8             # graphs per core
N_CORES = 8
P = 128
L1 = 508            # conv1 output length (510 - 3 + 1)
LP1 = 253           # after pool(3,2)
L2Y = 253           # conv2 (k=1) output length
L2Z = 252           # convc2 (k=2) output length
LF = 126            # after pool(2,2)

AF = mybir.ActivationFunctionType
ALU = mybir.AluOpType

_NC_CACHE = {}


def _build_nc(bench_loop=1):
    nc = bacc.Bacc("TRN2", target_bir_lowering=False, debug=False,
                   num_devices=N_CORES)

    # ---- DRAM parameters (per-core shapes) ----
    hT0_d = nc.declare_dram_parameter("hT0", [P, 2, GPC * NPAD], BF16, isOutput=False)
    AT_d = nc.declare_dram_parameter("AT", [GPC, P, 16, NPAD], FP8, isOutput=False)
    bind_d = nc.declare_dram_parameter("bindT", [P, 2, GPC * NPAD], BF16,
                                       isOutput=False)
    wcat_d = nc.declare_dram_parameter("Wcat", [P, 2, T * D], BF16, isOutput=False)
    wih_d = nc.declare_dram_parameter("WihT", [P, 2, 3 * D], BF16, isOutput=False)
    whh_d = nc.declare_dram_parameter("WhhT", [P, 2, 3 * D], BF16, isOutput=False)
    bias_d = nc.declare_dram_parameter("biases", [P, 22], FP32, isOutput=False)
    w12_d = nc.declare_dram_parameter("W12T", [P, 8, D], BF16, isOutput=False)
    wc_d = nc.declare_dram_parameter("WcT", [P, 20, 2 * D], BF16, isOutput=False)
    wyz_d = nc.declare_dram_parameter("wyzT", [P, 6, 1], BF16, isOutput=False)
    out_d = nc.declare_dram_parameter("out", [1, GPC], FP32, isOutput=True)

    with tile.TileContext(nc) as tc:
        with (
            tc.tile_pool(name="const", bufs=1) as cst,
            tc.tile_pool(name="state", bufs=1) as st,
            tc.tile_pool(name="atp", bufs=3) as atp,
            tc.tile_pool(name="mp", bufs=8) as mp,
            tc.tile_pool(name="rzp", bufs=8) as rzp,
            tc.tile_pool(name="gp", bufs=4) as gp,
            tc.tile_pool(name="cvp", bufs=5) as cvp,
            tc.tile_pool(name="psm", bufs=8, space="PSUM") as psm,
        ):
            # ---- constants ----
            wcat = cst.tile([P, 2, T * D], BF16)
            wih = cst.tile([P, 2, 3 * D], BF16)
            whh = cst.tile([P, 2, 3 * D], BF16)
            biases = cst.tile([P, 22], FP32)
            w12 = cst.tile([P, 8, D], BF16)
            wc = cst.tile([P, 20, 2 * D], BF16)
            wyz = cst.tile([P, 6, 1], BF16)

            # ---- per-graph state tiles + one-time setup (outside the
            # bench loop: initial h, slot-0 A^T, host-computed binds) ----
            hA, hB, ags, binds = [], [], [], []
            nc.sync.dma_start(wcat[:], wcat_d[:])
            for g in range(GPC):
                h0 = st.tile([P, 2, NPAD], BF16, tag=f"hA{g}", name=f"hA{g}")
                nc.sync.dma_start(h0[:], hT0_d[:, :, g * NPAD:(g + 1) * NPAD])
                hA.append(h0)
                hB.append(st.tile([P, 2, NPAD], BF16, tag=f"hB{g}", name=f"hB{g}"))
                ags.append(st.tile([P, 2, NPAD], BF16, tag=f"aT{g}", name=f"aT{g}"))
                bnd = st.tile([P, 2, NPAD], BF16, tag=f"bind{g}", name=f"bind{g}")
                nc.sync.dma_start(bnd[:], bind_d[:, :, g * NPAD:(g + 1) * NPAD])
                binds.append(bnd)
            at0 = st.tile([P, 16, NPAD], FP8, tag="at0", name="at0")
            nc.sync.dma_start(at0[:], AT_d[0])
            nc.sync.dma_start(wih[:], wih_d[:])
            nc.sync.dma_start(whh[:], whh_d[:])
            nc.sync.dma_start(biases[:], bias_d[:])
            nc.sync.dma_start(w12[:], w12_d[:])
            nc.sync.dma_start(wc[:], wc_d[:])
            nc.sync.dma_start(wyz[:], wyz_d[:])

            # The body is emitted in pairs with swapped hA/hB roles: a body
            # starting from X leaves h0 in Y (its mid-body feat load), which
            # is exactly the swapped body's initial h — no redundant reload,
            # and the loop back-edge seam is paid once per TWO executions.
            hint_engs = tuple(mybir.EngineType[e] for e in
                              ("PE", "DVE", "Activation", "SP", "Pool"))
            pairs, rem = bench_loop // 2, bench_loop % 2
            env = locals()
            if pairs:
                back_edge_label = "bench_backedge"
                with tc.For_i(0, pairs, 1,
                              staggered_reset=True,
                              hint_engines=hint_engs,
                              back_edge_label=back_edge_label) as loop_iv:
                    _kernel_body(nc, tc, env, swap=False, loop_hint=None)
                    # Re-arm the back-edge prefetch near the END of the pair:
                    # the loop-top hint's fetched block is evicted long before
                    # the branch in a body this large (~3.8us I$-stall).
                    _kernel_body(nc, tc, env, swap=True,
                                 loop_hint=(back_edge_label,
                                            loop_iv >= (pairs - 1), hint_engs))
            for _ in range(rem):
                _kernel_body(nc, tc, env, swap=False, loop_hint=None,
                             last=True)
            if not rem and not pairs:
                _kernel_body(nc, tc, env, swap=False, loop_hint=None,
                             last=True)

    nc.compile()
    return nc


def _kernel_body(nc, tc, env, swap=False, loop_hint=None, last=False):
    class E:
        pass
    e = E()
    e.__dict__.update(env)
    e.swap = swap
    e.loop_hint = loop_hint
    e.last = last
    _emit_body(nc, tc, e)


def _emit_body(nc, tc, e):
    (cst, st, atp, mp, rzp, gp, cvp, psm) = (
        e.cst, e.st, e.atp, e.mp, e.rzp, e.gp, e.cvp, e.psm)
    (wcat, wih, whh, biases, w12, wc, wyz) = (
        e.wcat, e.wih, e.whh, e.biases, e.w12, e.wc, e.wyz)
    (hT0_d, out_d, AT_d) = (e.hT0_d, e.out_d, e.AT_d)

    # Per-graph state tiles were created and initialized outside the bench
    # loop (_build_nc): bufX holds this body's initial h (hA on the first
    # body), at0 holds A^T_0, binds hold the b_lin correction. With 6 steps
    # the final h lands back in bufX and the mid-body feat load into bufY
    # doubles as the next (role-swapped) body's initial h.
    (ags, binds, at0) = (e.ags, e.binds, e.at0)
    bufX = e.hB if e.swap else e.hA
    bufY = e.hA if e.swap else e.hB

    # ================= GGNN: software-pipelined slots =================
    def emit_m(s, g):
        """m = h_g @ Wcat (node-major [node, T*D]); also prefetch A^T_g."""
        hg = (bufX if s % 2 == 0 else bufY)[g]
        if s == 0 and g == 0:
            atg = at0
        else:
            atg = atp.tile([P, 16, NPAD], FP8, tag="at")
            nc.sync.dma_start(atg[:], AT_d[g])
        m_tiles = []
        for i in range(4):
            msb = mp.tile([P, T, D], FP8, tag="m")
            for nt in range(2):
                pm = psm.tile([P, 512], FP32, tag="ps")
                for kt in range(2):
                    nc.tensor.matmul(
                        pm[:],
                        lhsT=hg[:, kt, i * P:(i + 1) * P],
                        rhs=wcat[:, kt, nt * 512:(nt + 1) * 512],
                        start=(kt == 0), stop=(kt == 1),
                    )
                dst = msb[:, nt * 2:(nt + 1) * 2, :]
                if (i * 2 + nt) % 8 < 5:
                    nc.vector.tensor_copy(dst, pm[:])
                else:
                    nc.scalar.activation(dst, pm[:], AF.Identity)
            m_tiles.append(msb)
        return m_tiles, atg

    def emit_agg(s, g, m_tiles, atg):
        """aT_g = m_stack^T @ A^T_g + bind_g.

        fp8 DoubleRow: each matmul contracts a (src-tile, type-pair) block
        256 deep (two stacked 128-row k-tiles), with both m and A^T in
        fp8e4 — ~1.4x the bf16 matmul stream rate.
        """
        pas = [psm.tile([P, 512], FP32, tag="ps", name=f"pa{mt}")
               for mt in range(2)]
        for c in range(8):
            i, tp = c // 2, c % 2
            for mt in range(2):
                nc.tensor.matmul(
                    pas[mt][:],
                    lhsT=m_tiles[i][:, 2 * tp:2 * tp + 2, mt * P:(mt + 1) * P],
                    rhs=atg[:, 4 * i + 2 * tp:4 * i + 2 * tp + 2, :],
                    start=(c == 0), stop=(c == 7),
                    perf_mode=mybir.MatmulPerfMode.DoubleRow,
                )
        for mt in range(2):
            nc.vector.scalar_tensor_tensor(
                ags[g][:, mt, :], pas[mt][:], 0.0, binds[g][:, mt, :],
                op0=ALU.add, op1=ALU.add)

    def emit_gru(s, g):
        hg = (bufX if s % 2 == 0 else bufY)[g]
        hn = (bufY if s % 2 == 0 else bufX)[g]
        agt = ags[g]
        rz_tiles = []
        for mt in range(4):
            pr = psm.tile([P, 512], FP32, tag="ps")
            for kt in range(2):
                nc.tensor.matmul(
                    pr[:], lhsT=wih[:, kt, mt * P:(mt + 1) * P],
                    rhs=agt[:, kt, :], start=(kt == 0), stop=False)
            for kt in range(2):
                nc.tensor.matmul(
                    pr[:], lhsT=whh[:, kt, mt * P:(mt + 1) * P],
                    rhs=hg[:, kt, :], start=False, stop=(kt == 1))
            rzt = rzp.tile([P, 512], BF16, tag="rz")
            nc.scalar.activation(rzt[:], pr[:], AF.Sigmoid,
                                 bias=biases[:, mt:mt + 1])
            rz_tiles.append(rzt)
        for mt in range(2):
            pi = psm.tile([P, 512], FP32, tag="ps")
            for kt in range(2):
                nc.tensor.matmul(
                    pi[:], lhsT=wih[:, kt, 2 * D + mt * P: 2 * D + (mt + 1) * P],
                    rhs=agt[:, kt, :], start=(kt == 0), stop=(kt == 1))
            ph = psm.tile([P, 512], FP32, tag="ps")
            for kt in range(2):
                nc.tensor.matmul(
                    ph[:], lhsT=whh[:, kt, 2 * D + mt * P: 2 * D + (mt + 1) * P],
                    rhs=hg[:, kt, :], start=(kt == 0), stop=(kt == 1))
            t1 = gp.tile([P, 512], BF16, tag="t1")
            nc.vector.scalar_tensor_tensor(
                t1[:], ph[:], biases[:, 6 + mt:7 + mt], rz_tiles[mt][:],
                op0=ALU.add, op1=ALU.mult)
            nc.vector.scalar_tensor_tensor(
                t1[:], pi[:], biases[:, 4 + mt:5 + mt], t1[:],
                op0=ALU.add, op1=ALU.add)
            nsb = gp.tile([P, 512], BF16, tag="nsb")
            nc.scalar.activation(nsb[:], t1[:], AF.Tanh)
            dsb = gp.tile([P, 512], BF16, tag="dsb")
            nc.vector.tensor_tensor(dsb[:], hg[:, mt, :], nsb[:],
                                    op=ALU.subtract)
            nc.vector.tensor_mul(dsb[:], rz_tiles[2 + mt][:], dsb[:])
            nc.vector.tensor_add(hn[:, mt, :], nsb[:], dsb[:])

    slots = [(s, g) for s in range(NUM_STEPS) for g in range(GPC)]
    nslot = len(slots)
    pend = {}
    for i, (s, g) in enumerate(slots):
        pend[i] = emit_m(s, g)
        if i >= 1:
            s1, g1 = slots[i - 1]
            mt_, at_ = pend.pop(i - 1)
            emit_agg(s1, g1, mt_, at_)
        if i >= 2:
            emit_gru(*slots[i - 2])
    mt_, at_ = pend.pop(nslot - 1)
    emit_agg(*slots[nslot - 1], mt_, at_)
    emit_gru(*slots[nslot - 2])
    emit_gru(*slots[nslot - 1])

    # feat reloads for the Z branch: reuse the bufY buffers (free after the
    # last step read them; NUM_STEPS is even so the final h lives in bufX).
    # This load is also the next (role-swapped) body's initial h.
    for g in range(GPC):
        nc.sync.dma_start(bufY[g][:], hT0_d[:, :, g * NPAD:(g + 1) * NPAD])

    hfin = bufX
    feats = bufY

    # ================= conv readout (pipelined across graphs) =================
    res = cst.tile([1, GPC], FP32)
    y1ps, z1ps, y2ps, z2ps = {}, {}, {}, {}

    def emit_Y1(g):
        hg = hfin[g]
        tiles = []
        for mt in range(2):
            pm = psm.tile([P, 512], FP32, tag="ps")
            first = True
            for k in range(3):
                for kt in range(2):
                    nc.tensor.matmul(
                        pm[:, :L1],
                        lhsT=w12[:, k * 2 + kt, mt * P:(mt + 1) * P],
                        rhs=hg[:, kt, k:k + L1],
                        start=first, stop=(k == 2 and kt == 1))
                    first = False
            y1 = cvp.tile([P, 512], BF16, tag="y1")
            if mt % 2 == 0:
                nc.vector.tensor_scalar(y1[:, :L1], pm[:, :L1],
                                        biases[:, 8 + mt:9 + mt], 0.0,
                                        op0=ALU.add, op1=ALU.max)
            else:
                nc.scalar.activation(y1[:, :L1], pm[:, :L1], AF.Relu,
                                     bias=biases[:, 8 + mt:9 + mt])
            yp = cvp.tile([P, LP1], BF16, tag="y1p")
            nc.vector.tensor_tensor(yp[:], y1[:, 0:505:2], y1[:, 1:506:2],
                                    op=ALU.max)
            nc.vector.tensor_tensor(yp[:], yp[:], y1[:, 2:507:2],
                                    op=ALU.max)
            tiles.append(yp)
        y1ps[g] = tiles

    def emit_Y2(g):
        y1p = y1ps.pop(g)
        tiles = []
        for mt in range(2):
            pm = psm.tile([P, 512], FP32, tag="ps")
            for kt in range(2):
                nc.tensor.matmul(
                    pm[:, :L2Y],
                    lhsT=w12[:, 6 + kt, mt * P:(mt + 1) * P],
                    rhs=y1p[kt][:],
                    start=(kt == 0), stop=(kt == 1))
            y2 = cvp.tile([P, L2Y], BF16, tag="y2")
            if mt % 2 == 0:
                nc.vector.tensor_scalar(y2[:], pm[:, :L2Y],
                                        biases[:, 10 + mt:11 + mt], 0.0,
                                        op0=ALU.add, op1=ALU.max)
            else:
                nc.scalar.activation(y2[:], pm[:, :L2Y], AF.Relu,
                                     bias=biases[:, 10 + mt:11 + mt])
            yp = cvp.tile([P, LF], BF16, tag="y2p")
            nc.vector.tensor_tensor(yp[:], y2[:, 0:251:2], y2[:, 1:252:2],
                                    op=ALU.max)
            tiles.append(yp)
        y2ps[g] = tiles

    def emit_Z1(g):
        hg = hfin[g]
        fg = feats[g]
        tiles = []
        for mt in range(4):
            pm = psm.tile([P, 512], FP32, tag="ps")
            first = True
            for k in range(3):
                for kt in range(4):
                    src = hg if kt < 2 else fg
                    nc.tensor.matmul(
                        pm[:, :L1],
                        lhsT=wc[:, k * 4 + kt, mt * P:(mt + 1) * P],
                        rhs=src[:, kt % 2, k:k + L1],
                        start=first, stop=(k == 2 and kt == 3))
                    first = False
            z1 = cvp.tile([P, 512], BF16, tag="z1")
            if mt % 2 == 0:
                nc.vector.tensor_scalar(z1[:, :L1], pm[:, :L1],
                                        biases[:, 12 + mt:13 + mt], 0.0,
                                        op0=ALU.add, op1=ALU.max)
            else:
                nc.scalar.activation(z1[:, :L1], pm[:, :L1], AF.Relu,
                                     bias=biases[:, 12 + mt:13 + mt])
            zp = cvp.tile([P, LP1], BF16, tag="z1p")
            nc.vector.tensor_tensor(zp[:], z1[:, 0:505:2], z1[:, 1:506:2],
                                    op=ALU.max)
            nc.vector.tensor_tensor(zp[:], zp[:], z1[:, 2:507:2],
                                    op=ALU.max)
            tiles.append(zp)
        z1ps[g] = tiles

    def emit_Z2(g):
        z1p = z1ps.pop(g)
        tiles = []
        for mt in range(4):
            pm = psm.tile([P, 512], FP32, tag="ps")
            first = True
            for k in range(2):
                for kt in range(4):
                    nc.tensor.matmul(
                        pm[:, :L2Z],
                        lhsT=wc[:, 12 + k * 4 + kt, mt * P:(mt + 1) * P],
                        rhs=z1p[kt][:, k:k + L2Z],
                        start=first, stop=(k == 1 and kt == 3))
                    first = False
            z2 = cvp.tile([P, L2Z], BF16, tag="z2")
            if mt % 2 == 0:
                nc.vector.tensor_scalar(z2[:], pm[:, :L2Z],
                                        biases[:, 16 + mt:17 + mt], 0.0,
                                        op0=ALU.add, op1=ALU.max)
            else:
                nc.scalar.activation(z2[:], pm[:, :L2Z], AF.Relu,
                                     bias=biases[:, 16 + mt:17 + mt])
            zp = cvp.tile([P, LF], BF16, tag="z2p")
            nc.vector.tensor_tensor(zp[:], z2[:, 0:251:2], z2[:, 1:252:2],
                                    op=ALU.max)
            tiles.append(zp)
        z2ps[g] = tiles

    def emit_fin(g):
        y2p = y2ps.pop(g)
        z2p = z2ps.pop(g)
        pv = psm.tile([P, 512], FP32, tag="ps")
        for kt in range(2):
            nc.tensor.matmul(pv[0:1, :LF], lhsT=wyz[:, kt, :],
                             rhs=y2p[kt][:], start=(kt == 0), stop=(kt == 1))
        ysb = cvp.tile([1, LF], FP32, tag="ysb")
        nc.scalar.activation(ysb[:], pv[0:1, :LF], AF.Identity,
                             bias=biases[0:1, 20:21])
        pz = psm.tile([P, 512], FP32, tag="ps")
        for kt in range(4):
            nc.tensor.matmul(pz[0:1, :LF], lhsT=wyz[:, 2 + kt, :],
                             rhs=z2p[kt][:], start=(kt == 0), stop=(kt == 3))
        zsb = cvp.tile([1, LF], FP32, tag="zsb")
        nc.scalar.activation(zsb[:], pz[0:1, :LF], AF.Identity,
                             bias=biases[0:1, 21:22])
        prod = cvp.tile([1, LF], FP32, tag="prod")
        nc.vector.tensor_mul(prod[:], ysb[:], zsb[:])
        ssum = cvp.tile([1, 1], FP32, tag="ssum")
        nc.vector.reduce_sum(ssum[:], prod[:], axis=mybir.AxisListType.X)
        nc.scalar.activation(res[:, g:g + 1], ssum[:], AF.Sigmoid,
                             scale=1.0 / LF)

    for g in range(GPC):
        emit_Y1(g)
        if g >= 1:
            emit_Y2(g - 1)
        emit_Z1(g)
        if g >= 1:
            emit_Z2(g - 1)
        if g >= 2:
            emit_fin(g - 2)
    if getattr(e, "loop_hint", None):
        label, hint, engs = e.loop_hint
        tc.mark_branch_hint_location(label, hint=hint, engines=engs)
    emit_Y2(GPC - 1)
    emit_Z2(GPC - 1)
    emit_fin(GPC - 2)
    emit_fin(GPC - 1)
    if not e.last:
        # slot-0 adjacency for the next body (its only reader ran long ago)
        nc.sync.dma_start(at0[:], AT_d[0])

    nc.sync.dma_start(out_d[:], res[:])


def _split_part(a, ntile):
    """[ntile*128, F...] -> [128, ntile, F...] with [p, t, ...] = a[t*128+p, ...]."""
    return np.ascontiguousarray(
        a.reshape(ntile, P, *a.shape[1:]).transpose(1, 0, *range(2, a.ndim + 1)))


def _prep_inputs(inputs):
    feat = np.asarray(inputs["feat"], np.float32)
    esrc = np.asarray(inputs["edge_src"]).astype(np.int64)
    edst = np.asarray(inputs["edge_dst"]).astype(np.int64)
    etyp = np.asarray(inputs["edge_type"]).astype(np.int64)

    # feature-major padded feat^T: per graph [256, 512]
    ftp = np.zeros((B, DIN, NPAD), np.float32)
    ftp[:, :, :N] = feat.transpose(0, 2, 1)

    # adjacency chunks: AT_all[g, c, s, d]; c = (src//128)*4 + t so that the
    # two types of a DoubleRow type-pair are adjacent in the c dim
    AT_all = np.zeros((B, 16, P, NPAD), np.float32)
    DT_all = np.zeros((B, T, NPAD), np.float32)
    g_of = esrc // N
    s_l = esrc % N
    d_l = edst % N
    np.add.at(AT_all, (g_of, (s_l // P) * 4 + etyp, s_l % P, d_l), 1.0)
    np.add.at(DT_all, (g_of, etyp, d_l), 1.0)

    W_lin = np.asarray(inputs["W_lin"], np.float32)
    Wcat = W_lin.transpose(2, 0, 1).reshape(D, T * D)
    b_lin = np.asarray(inputs["b_lin"], np.float32)
    # bind[g, f, dst] = sum_t b_lin[t, f] * indeg_t[g, dst]  (step-invariant
    # b_lin contribution to the aggregation, computed on host)
    bind_all = np.einsum('tf,gtn->gfn', b_lin, DT_all)
    W_ih = np.asarray(inputs["W_ih"], np.float32)
    W_hh = np.asarray(inputs["W_hh"], np.float32)
    b_ih = np.asarray(inputs["b_ih"], np.float32)
    b_hh = np.asarray(inputs["b_hh"], np.float32)

    def convT(w):  # [O, I, K] -> [128, K*ktiles, O]
        O, I, K = w.shape
        kt = I // P
        arr = w.transpose(2, 1, 0).reshape(K, kt, P, O).transpose(2, 0, 1, 3)
        return np.ascontiguousarray(arr.reshape(P, K * kt, O))

    biases = np.zeros((P, 22), np.float32)
    biases[:, 0:4] = (b_ih + b_hh)[:2 * D].reshape(4, P).T
    biases[:, 4:6] = b_ih[2 * D:].reshape(2, P).T
    biases[:, 6:8] = b_hh[2 * D:].reshape(2, P).T
    biases[:, 8:10] = np.asarray(inputs["conv1_b"], np.float32).reshape(2, P).T
    biases[:, 10:12] = np.asarray(inputs["conv2_b"], np.float32).reshape(2, P).T
    biases[:, 12:16] = np.asarray(inputs["convc1_b"], np.float32).reshape(4, P).T
    biases[:, 16:20] = np.asarray(inputs["convc2_b"], np.float32).reshape(4, P).T
    biases[0, 20] = float(np.asarray(inputs["mlp_y_b"])[0])
    biases[0, 21] = float(np.asarray(inputs["mlp_z_b"])[0])
    common = {
        "Wcat": _split_part(Wcat, 2).astype(bf16),
        "WihT": _split_part(np.ascontiguousarray(W_ih.T), 2).astype(bf16),
        "WhhT": _split_part(np.ascontiguousarray(W_hh.T), 2).astype(bf16),
        "biases": biases,
        "W12T": np.concatenate(
            [convT(np.asarray(inputs["conv1_w"], np.float32)),
             convT(np.asarray(inputs["conv2_w"], np.float32))], axis=1).astype(bf16),
        "WcT": np.concatenate(
            [convT(np.asarray(inputs["convc1_w"], np.float32)),
             convT(np.asarray(inputs["convc2_w"], np.float32))], axis=1).astype(bf16),
        "wyzT": np.concatenate(
            [_split_part(np.ascontiguousarray(np.asarray(inputs["mlp_y_w"], np.float32).T), 2),
             _split_part(np.ascontiguousarray(np.asarray(inputs["mlp_z_w"], np.float32).T), 4)],
            axis=1).astype(bf16),
    }

    in_maps = []
    for c in range(N_CORES):
        sl = slice(c * GPC, (c + 1) * GPC)
        hT0 = ftp[sl].transpose(1, 0, 2).reshape(DIN, GPC * NPAD)
        m = dict(common)
        hT0s = _split_part(hT0, 2)
        m["hT0"] = hT0s.astype(bf16)
        m["AT"] = np.ascontiguousarray(
            AT_all[sl].transpose(0, 2, 1, 3)).astype(fp8)
        m["bindT"] = _split_part(
            bind_all[sl].transpose(1, 0, 2).reshape(D, GPC * NPAD), 2
        ).astype(bf16)
        in_maps.append(m)
    return in_maps


def kernel(**inputs):
    if "nc" not in _NC_CACHE:
        _NC_CACHE["nc"] = _build_nc()
    nc = _NC_CACHE["nc"]
    in_maps = _prep_inputs(inputs)
    res = run_bass_kernel_spmd(nc, in_maps, list(range(N_CORES)))
    return np.concatenate([res.results[c]["out"][0] for c in range(N_CORES)])



# revision 46
# speedup vs baseline: 1.0699x; 1.0699x over previous
"""Devign model (GGNN + conv readout) Trainium2 kernel.

Data-parallel over the batch dim: 64 graphs -> 8 NeuronCores x 8 graphs.
Matmuls run in bf16 with fp32 PSUM accumulation, in a feature-major layout
([feature, node] on SBUF partitions) so no transposes are needed anywhere.
The GGNN scatter-add aggregation is reformulated as dense matmuls against
per-graph adjacency-count matrices A^T[(type,src), dst] built on the host.
The b_lin contribution (b_lin^T @ indeg, step-invariant) is computed on the
host and DMA'd once per graph at startup.

The aggregation contraction (44% of the GGNN matmul work) runs as
fp8e4m3 DoubleRow: A^T holds small integer counts (exact in fp8), the
per-edge-type messages m are evicted from PSUM as fp8, and each DoubleRow
instruction contracts a 256-deep (src-tile, type-pair) block at ~2x the
bf16 instruction throughput (measured 284ns vs 2x296ns per 512-wide MM).
Measured end-to-end rel-err 1.50e-2 of the 2e-2 budget; an error
simulation (errsim.py) shows extending fp8 to the m-matmul, GRU gates, or
the conv readout overshoots the budget, so those stay bf16.

The emission is software-pipelined across graphs (slot i emits m(i),
agg(i-1), gru(i-2)) so the PE always has independent matmuls to run while
the vector/scalar engines produce the SBUF tiles the next matmul group
needs; the conv readout is pipelined the same way. One-time setup
(initial h, slot-0 A^T, binds, all weights) is hoisted outside the
bench loop with next-iteration reloads emitted at body end, keeping the
back-edge seam short enough that the PE's HAM clock gate stays warm.
"""

import contextlib

import numpy as np
import ml_dtypes

import concourse.bass as bass
import concourse.bacc as bacc
import concourse.mybir as mybir
import concourse.tile as tile
from concourse.bass_utils import run_bass_kernel_spmd

bf16 = ml_dtypes.bfloat16
FP32 = mybir.dt.float32
BF16 = mybir.dt.bfloat16
FP8 = mybir.dt.float8e4
fp8 = ml_dtypes.float8_e4m3

# Problem constants (hardcoded per the spec).
B, N, DIN, D, T, NUM_STEPS = 64, 510, 256, 256, 4, 6
NPAD = 512          # padded nodes per graph
GPC = 8             # graphs per core
N_CORES = 8
P = 128
L1 = 508            # conv1 output length (510 - 3 + 1)
LP1 = 253           # after pool(3,2)
L2Y = 253           # conv2 (k=1) output length
L2Z = 252           # convc2 (k=2) output length
LF = 126            # after pool(2,2)

AF = mybir.ActivationFunctionType
ALU = mybir.AluOpType

# GRU r/z-gate matmuls run as fp8 DoubleRow for steps < RZ_DR_STEPS.
# Error-sim verdict: even a 1-3 step prefix costs ~2.2e-2 final rel-err
# (systematic gate-weight quantization bias, not per-step accumulation),
# which blows the 2e-2 budget on top of the aggregation-DR noise — so this
# stays 0 and only the aggregation runs in fp8.
RZ_DR_STEPS = 0

_NC_CACHE = {}


def _build_nc(bench_loop=1):
    nc = bacc.Bacc("TRN2", target_bir_lowering=False, debug=False,
                   num_devices=N_CORES)

    # ---- DRAM parameters (per-core shapes) ----
    hT0_d = nc.declare_dram_parameter("hT0", [P, 2, GPC * NPAD], BF16, isOutput=False)
    AT_d = nc.declare_dram_parameter("AT", [GPC, P, 16, NPAD], FP8, isOutput=False)
    bind_d = nc.declare_dram_parameter("bindT", [P, 2, GPC * NPAD], BF16,
                                       isOutput=False)
    wcat_d = nc.declare_dram_parameter("Wcat", [P, 2, T * D], BF16, isOutput=False)
    wih_d = nc.declare_dram_parameter("WihT", [P, 2, 3 * D], BF16, isOutput=False)
    whh_d = nc.declare_dram_parameter("WhhT", [P, 2, 3 * D], BF16, isOutput=False)
    bias_d = nc.declare_dram_parameter("biases", [P, 22], FP32, isOutput=False)
    w12_d = nc.declare_dram_parameter("W12T", [P, 8, D], BF16, isOutput=False)
    wc_d = nc.declare_dram_parameter("WcT", [P, 20, 2 * D], BF16, isOutput=False)
    wyz_d = nc.declare_dram_parameter("wyzT", [P, 6, 1], BF16, isOutput=False)
    if RZ_DR_STEPS > 0:
        ff8_d = nc.declare_dram_parameter("featF8", [P, 2, GPC * NPAD], FP8,
                                          isOutput=False)
        wih8_d = nc.declare_dram_parameter("WihF8", [P, 2, 2 * D], FP8,
                                           isOutput=False)
        whh8_d = nc.declare_dram_parameter("WhhF8", [P, 2, 2 * D], FP8,
                                           isOutput=False)
    out_d = nc.declare_dram_parameter("out", [1, GPC], FP32, isOutput=True)

    with tile.TileContext(nc) as tc:
        with (
            tc.tile_pool(name="const", bufs=1) as cst,
            tc.tile_pool(name="state", bufs=1) as st,
            tc.tile_pool(name="atp", bufs=3) as atp,
            tc.tile_pool(name="mp", bufs=8) as mp,
            tc.tile_pool(name="rzp", bufs=8) as rzp,
            tc.tile_pool(name="gp", bufs=4) as gp,
            tc.tile_pool(name="cvp", bufs=5) as cvp,
            tc.tile_pool(name="psm", bufs=8, space="PSUM") as psm,
        ):
            # ---- constants ----
            wcat = cst.tile([P, 2, T * D], BF16)
            wih = cst.tile([P, 2, 3 * D], BF16)
            whh = cst.tile([P, 2, 3 * D], BF16)
            biases = cst.tile([P, 22], FP32)
            w12 = cst.tile([P, 8, D], BF16)
            wc = cst.tile([P, 20, 2 * D], BF16)
            wyz = cst.tile([P, 6, 1], BF16)
            if RZ_DR_STEPS > 0:
                wih8 = cst.tile([P, 2, 2 * D], FP8)
                whh8 = cst.tile([P, 2, 2 * D], FP8)
            else:
                wih8 = whh8 = None

            # ---- per-graph state tiles + one-time setup (outside the
            # bench loop: initial h, slot-0 A^T, host-computed binds) ----
            hA, hB, ags, binds, hshs, ags8 = [], [], [], [], [], []
            nc.sync.dma_start(wcat[:], wcat_d[:])
            for g in range(GPC):
                h0 = st.tile([P, 2, NPAD], BF16, tag=f"hA{g}", name=f"hA{g}")
                nc.sync.dma_start(h0[:], hT0_d[:, :, g * NPAD:(g + 1) * NPAD])
                hA.append(h0)
                hB.append(st.tile([P, 2, NPAD], BF16, tag=f"hB{g}", name=f"hB{g}"))
                ags.append(st.tile([P, 2, NPAD], BF16, tag=f"aT{g}", name=f"aT{g}"))
                bnd = st.tile([P, 2, NPAD], BF16, tag=f"bind{g}", name=f"bind{g}")
                nc.sync.dma_start(bnd[:], bind_d[:, :, g * NPAD:(g + 1) * NPAD])
                binds.append(bnd)
                if RZ_DR_STEPS > 0:
                    hsh = st.tile([P, 2, NPAD], FP8, tag=f"hsh{g}", name=f"hsh{g}")
                    nc.sync.dma_start(hsh[:], ff8_d[:, :, g * NPAD:(g + 1) * NPAD])
                    hshs.append(hsh)
                    ags8.append(st.tile([P, 2, NPAD], FP8, tag=f"a8T{g}",
                                        name=f"a8T{g}"))
            at0 = st.tile([P, 16, NPAD], FP8, tag="at0", name="at0")
            nc.sync.dma_start(at0[:], AT_d[0])
            nc.sync.dma_start(wih[:], wih_d[:])
            nc.sync.dma_start(whh[:], whh_d[:])
            if RZ_DR_STEPS > 0:
                nc.sync.dma_start(wih8[:], wih8_d[:])
                nc.sync.dma_start(whh8[:], whh8_d[:])
            nc.sync.dma_start(biases[:], bias_d[:])
            nc.sync.dma_start(w12[:], w12_d[:])
            nc.sync.dma_start(wc[:], wc_d[:])
            nc.sync.dma_start(wyz[:], wyz_d[:])

            # The body is emitted in pairs with swapped hA/hB roles: a body
            # starting from X leaves h0 in Y (its mid-body feat load), which
            # is exactly the swapped body's initial h — no redundant reload,
            # and the loop back-edge seam is paid once per TWO executions.
            hint_engs = tuple(mybir.EngineType[e] for e in
                              ("PE", "DVE", "Activation", "SP", "Pool"))
            pairs, rem = bench_loop // 2, bench_loop % 2
            env = locals()
            if pairs:
                back_edge_label = "bench_backedge"
                with tc.For_i(0, pairs, 1,
                              staggered_reset=True,
                              hint_engines=hint_engs,
                              back_edge_label=back_edge_label) as loop_iv:
                    _kernel_body(nc, tc, env, swap=False, loop_hint=None)
                    # Re-arm the back-edge prefetch near the END of the pair:
                    # the loop-top hint's fetched block is evicted long before
                    # the branch in a body this large (~3.8us I$-stall).
                    _kernel_body(nc, tc, env, swap=True,
                                 loop_hint=(back_edge_label,
                                            loop_iv >= (pairs - 1), hint_engs))
            for _ in range(rem):
                _kernel_body(nc, tc, env, swap=False, loop_hint=None,
                             last=True)
            if not rem and not pairs:
                _kernel_body(nc, tc, env, swap=False, loop_hint=None,
                             last=True)

    nc.compile()
    return nc


def _kernel_body(nc, tc, env, swap=False, loop_hint=None, last=False):
    class E:
        pass
    e = E()
    e.__dict__.update(env)
    e.swap = swap
    e.loop_hint = loop_hint
    e.last = last
    _emit_body(nc, tc, e)


def _emit_body(nc, tc, e):
    (cst, st, atp, mp, rzp, gp, cvp, psm) = (
        e.cst, e.st, e.atp, e.mp, e.rzp, e.gp, e.cvp, e.psm)
    (wcat, wih, whh, biases, w12, wc, wyz) = (
        e.wcat, e.wih, e.whh, e.biases, e.w12, e.wc, e.wyz)
    (hT0_d, out_d, AT_d) = (e.hT0_d, e.out_d, e.AT_d)

    # Per-graph state tiles were created and initialized outside the bench
    # loop (_build_nc): bufX holds this body's initial h (hA on the first
    # body), at0 holds A^T_0, binds hold the b_lin correction. With 6 steps
    # the final h lands back in bufX and the mid-body feat load into bufY
    # doubles as the next (role-swapped) body's initial h.
    (ags, binds, at0) = (e.ags, e.binds, e.at0)
    (hshs, ags8, wih8, whh8) = (e.hshs, e.ags8, e.wih8, e.whh8)
    ff8_d = e.ff8_d if RZ_DR_STEPS > 0 else None
    bufX = e.hB if e.swap else e.hA
    bufY = e.hA if e.swap else e.hB

    # ================= GGNN: software-pipelined slots =================
    def emit_m(s, g):
        """m = h_g @ Wcat (node-major [node, T*D]); also prefetch A^T_g."""
        hg = (bufX if s % 2 == 0 else bufY)[g]
        if s == 0 and g == 0:
            atg = at0
        else:
            atg = atp.tile([P, 16, NPAD], FP8, tag="at")
            nc.sync.dma_start(atg[:], AT_d[g])
        m_tiles = []
        for i in range(4):
            msb = mp.tile([P, T, D], FP8, tag="m")
            for nt in range(2):
                pm = psm.tile([P, 512], FP32, tag="ps")
                for kt in range(2):
                    nc.tensor.matmul(
                        pm[:],
                        lhsT=hg[:, kt, i * P:(i + 1) * P],
                        rhs=wcat[:, kt, nt * 512:(nt + 1) * 512],
                        start=(kt == 0), stop=(kt == 1),
                    )
                dst = msb[:, nt * 2:(nt + 1) * 2, :]
                if (i * 2 + nt) % 8 < 5:
                    nc.vector.tensor_copy(dst, pm[:])
                else:
                    nc.scalar.activation(dst, pm[:], AF.Identity)
            m_tiles.append(msb)
        return m_tiles, atg

    def emit_agg(s, g, m_tiles, atg):
        """aT_g = m_stack^T @ A^T_g + bind_g.

        fp8 DoubleRow: each matmul contracts a (src-tile, type-pair) block
        256 deep (two stacked 128-row k-tiles), with both m and A^T in
        fp8e4 — ~1.4x the bf16 matmul stream rate.
        """
        pas = [psm.tile([P, 512], FP32, tag="ps", name=f"pa{mt}")
               for mt in range(2)]
        for c in range(8):
            i, tp = c // 2, c % 2
            for mt in range(2):
                nc.tensor.matmul(
                    pas[mt][:],
                    lhsT=m_tiles[i][:, 2 * tp:2 * tp + 2, mt * P:(mt + 1) * P],
                    rhs=atg[:, 4 * i + 2 * tp:4 * i + 2 * tp + 2, :],
                    start=(c == 0), stop=(c == 7),
                    perf_mode=mybir.MatmulPerfMode.DoubleRow,
                )
        for mt in range(2):
            nc.vector.scalar_tensor_tensor(
                ags[g][:, mt, :], pas[mt][:], 0.0, binds[g][:, mt, :],
                op0=ALU.add, op1=ALU.add)
            if s < RZ_DR_STEPS:
                # fp8 copy of a for the DoubleRow r/z matmuls, on the
                # otherwise-idle Pool engine
                nc.gpsimd.tensor_copy(ags8[g][:, mt, :], ags[g][:, mt, :])

    def emit_gru(s, g):
        hg = (bufX if s % 2 == 0 else bufY)[g]
        hn = (bufY if s % 2 == 0 else bufX)[g]
        agt = ags[g]
        rz_tiles = []
        for mt in range(4):
            pr = psm.tile([P, 512], FP32, tag="ps")
            if s < RZ_DR_STEPS:
                nc.tensor.matmul(
                    pr[:], lhsT=wih8[:, :, mt * P:(mt + 1) * P],
                    rhs=ags8[g][:, :, :], start=True, stop=False,
                    perf_mode=mybir.MatmulPerfMode.DoubleRow)
                nc.tensor.matmul(
                    pr[:], lhsT=whh8[:, :, mt * P:(mt + 1) * P],
                    rhs=hshs[g][:, :, :], start=False, stop=True,
                    perf_mode=mybir.MatmulPerfMode.DoubleRow)
            else:
                for kt in range(2):
                    nc.tensor.matmul(
                        pr[:], lhsT=wih[:, kt, mt * P:(mt + 1) * P],
                        rhs=agt[:, kt, :], start=(kt == 0), stop=False)
                for kt in range(2):
                    nc.tensor.matmul(
                        pr[:], lhsT=whh[:, kt, mt * P:(mt + 1) * P],
                        rhs=hg[:, kt, :], start=False, stop=(kt == 1))
            rzt = rzp.tile([P, 512], BF16, tag="rz")
            nc.scalar.activation(rzt[:], pr[:], AF.Sigmoid,
                                 bias=biases[:, mt:mt + 1])
            rz_tiles.append(rzt)
        for mt in range(2):
            pi = psm.tile([P, 512], FP32, tag="ps")
            for kt in range(2):
                nc.tensor.matmul(
                    pi[:], lhsT=wih[:, kt, 2 * D + mt * P: 2 * D + (mt + 1) * P],
                    rhs=agt[:, kt, :], start=(kt == 0), stop=(kt == 1))
            ph = psm.tile([P, 512], FP32, tag="ps")
            for kt in range(2):
                nc.tensor.matmul(
                    ph[:], lhsT=whh[:, kt, 2 * D + mt * P: 2 * D + (mt + 1) * P],
                    rhs=hg[:, kt, :], start=(kt == 0), stop=(kt == 1))
            t1 = gp.tile([P, 512], BF16, tag="t1")
            nc.vector.scalar_tensor_tensor(
                t1[:], ph[:], biases[:, 6 + mt:7 + mt], rz_tiles[mt][:],
                op0=ALU.add, op1=ALU.mult)
            nc.vector.scalar_tensor_tensor(
                t1[:], pi[:], biases[:, 4 + mt:5 + mt], t1[:],
                op0=ALU.add, op1=ALU.add)
            nsb = gp.tile([P, 512], BF16, tag="nsb")
            nc.scalar.activation(nsb[:], t1[:], AF.Tanh)
            dsb = gp.tile([P, 512], BF16, tag="dsb")
            nc.vector.tensor_tensor(dsb[:], hg[:, mt, :], nsb[:],
                                    op=ALU.subtract)
            nc.vector.tensor_mul(dsb[:], rz_tiles[2 + mt][:], dsb[:])
            nc.vector.tensor_add(hn[:, mt, :], nsb[:], dsb[:])
            if s + 1 < RZ_DR_STEPS:
                # fp8 shadow of h_{s+1} for the next step's DR r/z matmuls
                nc.gpsimd.tensor_copy(hshs[g][:, mt, :], hn[:, mt, :])

    slots = [(s, g) for s in range(NUM_STEPS) for g in range(GPC)]
    nslot = len(slots)
    pend = {}
    for i, (s, g) in enumerate(slots):
        pend[i] = emit_m(s, g)
        if i >= 1:
            s1, g1 = slots[i - 1]
            mt_, at_ = pend.pop(i - 1)
            emit_agg(s1, g1, mt_, at_)
        if i >= 2:
            emit_gru(*slots[i - 2])
    mt_, at_ = pend.pop(nslot - 1)
    emit_agg(*slots[nslot - 1], mt_, at_)
    emit_gru(*slots[nslot - 2])
    emit_gru(*slots[nslot - 1])

    # feat reloads for the Z branch: reuse the bufY buffers (free after the
    # last step read them; NUM_STEPS is even so the final h lives in bufX).
    # This load is also the next (role-swapped) body's initial h.
    for g in range(GPC):
        nc.sync.dma_start(bufY[g][:], hT0_d[:, :, g * NPAD:(g + 1) * NPAD])
        if RZ_DR_STEPS > 0:
            # reset the fp8 h shadow to feat for the next body's step 0
            nc.sync.dma_start(hshs[g][:], ff8_d[:, :, g * NPAD:(g + 1) * NPAD])

    hfin = bufX
    feats = bufY

    # ================= conv readout (pipelined across graphs) =================
    res = cst.tile([1, GPC], FP32)
    y1ps, z1ps, y2ps, z2ps = {}, {}, {}, {}

    def emit_Y1(g):
        hg = hfin[g]
        tiles = []
        for mt in range(2):
            pm = psm.tile([P, 512], FP32, tag="ps")
            first = True
            for k in range(3):
                for kt in range(2):
                    nc.tensor.matmul(
                        pm[:, :L1],
                        lhsT=w12[:, k * 2 + kt, mt * P:(mt + 1) * P],
                        rhs=hg[:, kt, k:k + L1],
                        start=first, stop=(k == 2 and kt == 1))
                    first = False
            y1 = cvp.tile([P, 512], BF16, tag="y1")
            if mt % 2 == 0:
                nc.vector.tensor_scalar(y1[:, :L1], pm[:, :L1],
                                        biases[:, 8 + mt:9 + mt], 0.0,
                                        op0=ALU.add, op1=ALU.max)
            else:
                nc.scalar.activation(y1[:, :L1], pm[:, :L1], AF.Relu,
                                     bias=biases[:, 8 + mt:9 + mt])
            yp = cvp.tile([P, LP1], BF16, tag="y1p")
            nc.vector.tensor_tensor(yp[:], y1[:, 0:505:2], y1[:, 1:506:2],
                                    op=ALU.max)
            nc.vector.tensor_tensor(yp[:], yp[:], y1[:, 2:507:2],
                                    op=ALU.max)
            tiles.append(yp)
        y1ps[g] = tiles

    def emit_Y2(g):
        y1p = y1ps.pop(g)
        tiles = []
        for mt in range(2):
            pm = psm.tile([P, 512], FP32, tag="ps")
            for kt in range(2):
                nc.tensor.matmul(
                    pm[:, :L2Y],
                    lhsT=w12[:, 6 + kt, mt * P:(mt + 1) * P],
                    rhs=y1p[kt][:],
                    start=(kt == 0), stop=(kt == 1))
            y2 = cvp.tile([P, L2Y], BF16, tag="y2")
            if mt % 2 == 0:
                nc.vector.tensor_scalar(y2[:], pm[:, :L2Y],
                                        biases[:, 10 + mt:11 + mt], 0.0,
                                        op0=ALU.add, op1=ALU.max)
            else:
                nc.scalar.activation(y2[:], pm[:, :L2Y], AF.Relu,
                                     bias=biases[:, 10 + mt:11 + mt])
            yp = cvp.tile([P, LF], BF16, tag="y2p")
            nc.vector.tensor_tensor(yp[:], y2[:, 0:251:2], y2[:, 1:252:2],
                                    op=ALU.max)
            tiles.append(yp)
        y2ps[g] = tiles

    def emit_Z1(g):
        hg = hfin[g]
        fg = feats[g]
        tiles = []
        for mt in range(4):
            pm = psm.tile([P, 512], FP32, tag="ps")
            first = True
            for k in range(3):
                for kt in range(4):
                    src = hg if kt < 2 else fg
                    nc.tensor.matmul(
                        pm[:, :L1],
                        lhsT=wc[:, k * 4 + kt, mt * P:(mt + 1) * P],
                        rhs=src[:, kt % 2, k:k + L1],
                        start=first, stop=(k == 2 and kt == 3))
                    first = False
            z1 = cvp.tile([P, 512], BF16, tag="z1")
            if mt % 2 == 0:
                nc.vector.tensor_scalar(z1[:, :L1], pm[:, :L1],
                                        biases[:, 12 + mt:13 + mt], 0.0,
                                        op0=ALU.add, op1=ALU.max)
            else:
                nc.scalar.activation(z1[:, :L1], pm[:, :L1], AF.Relu,
                                     bias=biases[:, 12 + mt:13 + mt])
            zp = cvp.tile([P, LP1], BF16, tag="z1p")
            nc.vector.tensor_tensor(zp[:], z1[:, 0:505:2], z1[:, 1:506:2],
                                    op=ALU.max)
            nc.vector.tensor_tensor(zp[:], zp[:], z1[:, 2:507:2],
                                    op=ALU.max)
            tiles.append(zp)
        z1ps[g] = tiles

    def emit_Z2(g):
        z1p = z1ps.pop(g)
        tiles = []
        for mt in range(4):
            pm = psm.tile([P, 512], FP32, tag="ps")
            first = True
            for k in range(2):
                for kt in range(4):
                    nc.tensor.matmul(
                        pm[:, :L2Z],
                        lhsT=wc[:, 12 + k * 4 + kt, mt * P:(mt + 1) * P],
                        rhs=z1p[kt][:, k:k + L2Z],
                        start=first, stop=(k == 1 and kt == 3))
                    first = False
            z2 = cvp.tile([P, L2Z], BF16, tag="z2")
            if mt % 2 == 0:
                nc.vector.tensor_scalar(z2[:], pm[:, :L2Z],
                                        biases[:, 16 + mt:17 + mt], 0.0,
                                        op0=ALU.add, op1=ALU.max)
            else:
                nc.scalar.activation(z2[:], pm[:, :L2Z], AF.Relu,
                                     bias=biases[:, 16 + mt:17 + mt])
            zp = cvp.tile([P, LF], BF16, tag="z2p")
            nc.vector.tensor_tensor(zp[:], z2[:, 0:251:2], z2[:, 1:252:2],
                                    op=ALU.max)
            tiles.append(zp)
        z2ps[g] = tiles

    def emit_fin(g):
        y2p = y2ps.pop(g)
        z2p = z2ps.pop(g)
        pv = psm.tile([P, 512], FP32, tag="ps")
        for kt in range(2):
            nc.tensor.matmul(pv[0:1, :LF], lhsT=wyz[:, kt, :],
                             rhs=y2p[kt][:], start=(kt == 0), stop=(kt == 1))
        ysb = cvp.tile([1, LF], FP32, tag="ysb")
        nc.scalar.activation(ysb[:], pv[0:1, :LF], AF.Identity,
                             bias=biases[0:1, 20:21])
        pz = psm.tile([P, 512], FP32, tag="ps")
        for kt in range(4):
            nc.tensor.matmul(pz[0:1, :LF], lhsT=wyz[:, 2 + kt, :],
                             rhs=z2p[kt][:], start=(kt == 0), stop=(kt == 3))
        zsb = cvp.tile([1, LF], FP32, tag="zsb")
        nc.scalar.activation(zsb[:], pz[0:1, :LF], AF.Identity,
                             bias=biases[0:1, 21:22])
        prod = cvp.tile([1, LF], FP32, tag="prod")
        nc.vector.tensor_mul(prod[:], ysb[:], zsb[:])
        ssum = cvp.tile([1, 1], FP32, tag="ssum")
        nc.vector.reduce_sum(ssum[:], prod[:], axis=mybir.AxisListType.X)
        nc.scalar.activation(res[:, g:g + 1], ssum[:], AF.Sigmoid,
                             scale=1.0 / LF)

    for g in range(GPC):
        emit_Y1(g)
        if g >= 1:
            emit_Y2(g - 1)
        emit_Z1(g)
        if g >= 1:
            emit_Z2(g - 1)
        if g >= 2:
            emit_fin(g - 2)
    if getattr(e, "loop_hint", None):
        label, hint, engs = e.loop_hint
        tc.mark_branch_hint_location(label, hint=hint, engines=engs)
    emit_Y2(GPC - 1)
    emit_Z2(GPC - 1)
    emit_fin(GPC - 2)
    emit_fin(GPC - 1)
    if not e.last:
        # slot-0 adjacency for the next body (its only reader ran long ago)
        nc.sync.dma_start(at0[:], AT_d[0])

    nc.sync.dma_start(out_d[:], res[:])


def _split_part(a, ntile):
    """[ntile*128, F...] -> [128, ntile, F...] with [p, t, ...] = a[t*128+p, ...]."""
    return np.ascontiguousarray(
        a.reshape(ntile, P, *a.shape[1:]).transpose(1, 0, *range(2, a.ndim + 1)))


def _prep_inputs(inputs):
    feat = np.asarray(inputs["feat"], np.float32)
    esrc = np.asarray(inputs["edge_src"]).astype(np.int64)
    edst = np.asarray(inputs["edge_dst"]).astype(np.int64)
    etyp = np.asarray(inputs["edge_type"]).astype(np.int64)

    # feature-major padded feat^T: per graph [256, 512]
    ftp = np.zeros((B, DIN, NPAD), np.float32)
    ftp[:, :, :N] = feat.transpose(0, 2, 1)

    # adjacency chunks: AT_all[g, c, s, d]; c = (src//128)*4 + t so that the
    # two types of a DoubleRow type-pair are adjacent in the c dim
    AT_all = np.zeros((B, 16, P, NPAD), np.float32)
    DT_all = np.zeros((B, T, NPAD), np.float32)
    g_of = esrc // N
    s_l = esrc % N
    d_l = edst % N
    np.add.at(AT_all, (g_of, (s_l // P) * 4 + etyp, s_l % P, d_l), 1.0)
    np.add.at(DT_all, (g_of, etyp, d_l), 1.0)

    W_lin = np.asarray(inputs["W_lin"], np.float32)
    Wcat = W_lin.transpose(2, 0, 1).reshape(D, T * D)
    b_lin = np.asarray(inputs["b_lin"], np.float32)
    # bind[g, f, dst] = sum_t b_lin[t, f] * indeg_t[g, dst]  (step-invariant
    # b_lin contribution to the aggregation, computed on host)
    bind_all = np.einsum('tf,gtn->gfn', b_lin, DT_all)
    W_ih = np.asarray(inputs["W_ih"], np.float32)
    W_hh = np.asarray(inputs["W_hh"], np.float32)
    b_ih = np.asarray(inputs["b_ih"], np.float32)
    b_hh = np.asarray(inputs["b_hh"], np.float32)

    def convT(w):  # [O, I, K] -> [128, K*ktiles, O]
        O, I, K = w.shape
        kt = I // P
        arr = w.transpose(2, 1, 0).reshape(K, kt, P, O).transpose(2, 0, 1, 3)
        return np.ascontiguousarray(arr.reshape(P, K * kt, O))

    biases = np.zeros((P, 22), np.float32)
    biases[:, 0:4] = (b_ih + b_hh)[:2 * D].reshape(4, P).T
    biases[:, 4:6] = b_ih[2 * D:].reshape(2, P).T
    biases[:, 6:8] = b_hh[2 * D:].reshape(2, P).T
    biases[:, 8:10] = np.asarray(inputs["conv1_b"], np.float32).reshape(2, P).T
    biases[:, 10:12] = np.asarray(inputs["conv2_b"], np.float32).reshape(2, P).T
    biases[:, 12:16] = np.asarray(inputs["convc1_b"], np.float32).reshape(4, P).T
    biases[:, 16:20] = np.asarray(inputs["convc2_b"], np.float32).reshape(4, P).T
    biases[0, 20] = float(np.asarray(inputs["mlp_y_b"])[0])
    biases[0, 21] = float(np.asarray(inputs["mlp_z_b"])[0])
    WihTs = _split_part(np.ascontiguousarray(W_ih.T), 2)
    WhhTs = _split_part(np.ascontiguousarray(W_hh.T), 2)
    common = {
        "Wcat": _split_part(Wcat, 2).astype(bf16),
        "WihT": WihTs.astype(bf16),
        "WhhT": WhhTs.astype(bf16),
        "biases": biases,
        "W12T": np.concatenate(
            [convT(np.asarray(inputs["conv1_w"], np.float32)),
             convT(np.asarray(inputs["conv2_w"], np.float32))], axis=1).astype(bf16),
        "WcT": np.concatenate(
            [convT(np.asarray(inputs["convc1_w"], np.float32)),
             convT(np.asarray(inputs["convc2_w"], np.float32))], axis=1).astype(bf16),
        "wyzT": np.concatenate(
            [_split_part(np.ascontiguousarray(np.asarray(inputs["mlp_y_w"], np.float32).T), 2),
             _split_part(np.ascontiguousarray(np.asarray(inputs["mlp_z_w"], np.float32).T), 4)],
            axis=1).astype(bf16),
    }
    if RZ_DR_STEPS > 0:
        common["WihF8"] = np.ascontiguousarray(
            WihTs[:, :, :2 * D]).astype(fp8)
        common["WhhF8"] = np.ascontiguousarray(
            WhhTs[:, :, :2 * D]).astype(fp8)

    in_maps = []
    for c in range(N_CORES):
        sl = slice(c * GPC, (c + 1) * GPC)
        hT0 = ftp[sl].transpose(1, 0, 2).reshape(DIN, GPC * NPAD)
        m = dict(common)
        hT0s = _split_part(hT0, 2)
        m["hT0"] = hT0s.astype(bf16)
        if RZ_DR_STEPS > 0:
            m["featF8"] = hT0s.astype(fp8)
        m["AT"] = np.ascontiguousarray(
            AT_all[sl].transpose(0, 2, 1, 3)).astype(fp8)
        m["bindT"] = _split_part(
            bind_all[sl].transpose(1, 0, 2).reshape(D, GPC * NPAD), 2
        ).astype(bf16)
        in_maps.append(m)
    return in_maps


def kernel(**inputs):
    if "nc" not in _NC_CACHE:
        _NC_CACHE["nc"] = _build_nc()
    nc = _NC_CACHE["nc"]
    in_maps = _prep_inputs(inputs)
    res = run_bass_kernel_spmd(nc, in_maps, list(range(N_CORES)))
    return np.concatenate([res.results[c]["out"][0] for c in range(N_CORES)])



# revision 47
# speedup vs baseline: 1.0833x; 1.0126x over previous
"""Devign model (GGNN + conv readout) Trainium2 kernel.

Data-parallel over the batch dim: 64 graphs -> 8 NeuronCores x 8 graphs.
Matmuls run in bf16 with fp32 PSUM accumulation, in a feature-major layout
([feature, node] on SBUF partitions) so no transposes are needed anywhere.
The GGNN scatter-add aggregation is reformulated as dense matmuls against
per-graph adjacency-count matrices A^T[(type,src), dst] built on the host.
The b_lin contribution (b_lin^T @ indeg, step-invariant) is computed on the
host and DMA'd once per graph at startup.

The aggregation contraction (44% of the GGNN matmul work) runs as
fp8e4m3 DoubleRow: A^T holds small integer counts (exact in fp8), the
per-edge-type messages m are evicted from PSUM as fp8, and each DoubleRow
instruction contracts a 256-deep (src-tile, type-pair) block at ~2x the
bf16 instruction throughput (measured 284ns vs 2x296ns per 512-wide MM).
Measured end-to-end rel-err 1.50e-2 of the 2e-2 budget; an error
simulation (errsim.py) shows extending fp8 to the m-matmul, GRU gates, or
the conv readout overshoots the budget, so those stay bf16.

The emission is software-pipelined across graphs (slot i emits m(i),
agg(i-1), gru(i-2)) so the PE always has independent matmuls to run while
the vector/scalar engines produce the SBUF tiles the next matmul group
needs; the conv readout is pipelined the same way. One-time setup
(initial h, slot-0 A^T, binds, all weights) is hoisted outside the
bench loop with next-iteration reloads emitted at body end, keeping the
back-edge seam short enough that the PE's HAM clock gate stays warm.
"""

import contextlib

import numpy as np
import ml_dtypes

import concourse.bass as bass
import concourse.bacc as bacc
import concourse.mybir as mybir
import concourse.tile as tile
from concourse.bass_utils import run_bass_kernel_spmd

bf16 = ml_dtypes.bfloat16
FP32 = mybir.dt.float32
BF16 = mybir.dt.bfloat16
FP8 = mybir.dt.float8e4
fp8 = ml_dtypes.float8_e4m3

# Problem constants (hardcoded per the spec).
B, N, DIN, D, T, NUM_STEPS = 64, 510, 256, 256, 4, 6
NPAD = 512          # padded nodes per graph
GPC = 8             # graphs per core
N_CORES = 8
P = 128
L1 = 508            # conv1 output length (510 - 3 + 1)
LP1 = 253           # after pool(3,2)
L2Y = 253           # conv2 (k=1) output length
L2Z = 252           # convc2 (k=2) output length
LF = 126            # after pool(2,2)

AF = mybir.ActivationFunctionType
ALU = mybir.AluOpType

# GRU r/z-gate matmuls run as fp8 DoubleRow for steps < RZ_DR_STEPS.
# Error-sim verdict: even a 1-3 step prefix costs ~2.2e-2 final rel-err
# (systematic gate-weight quantization bias, not per-step accumulation),
# which blows the 2e-2 budget on top of the aggregation-DR noise — so this
# stays 0 and only the aggregation runs in fp8.
RZ_DR_STEPS = 0

_NC_CACHE = {}


def _build_nc(bench_loop=1):
    nc = bacc.Bacc("TRN2", target_bir_lowering=False, debug=False,
                   num_devices=N_CORES)

    # ---- DRAM parameters (per-core shapes) ----
    hT0_d = nc.declare_dram_parameter("hT0", [P, 2, GPC * NPAD], BF16, isOutput=False)
    AT_d = nc.declare_dram_parameter("AT", [GPC, P, 16, NPAD], FP8, isOutput=False)
    bind_d = nc.declare_dram_parameter("bindT", [P, 2, GPC * NPAD], BF16,
                                       isOutput=False)
    wcat_d = nc.declare_dram_parameter("Wcat", [P, 2, T * D], BF16, isOutput=False)
    wih_d = nc.declare_dram_parameter("WihT", [P, 2, 3 * D], BF16, isOutput=False)
    whh_d = nc.declare_dram_parameter("WhhT", [P, 2, 3 * D], BF16, isOutput=False)
    bias_d = nc.declare_dram_parameter("biases", [P, 22], FP32, isOutput=False)
    w12_d = nc.declare_dram_parameter("W12T", [P, 8, D], BF16, isOutput=False)
    wc_d = nc.declare_dram_parameter("WcT", [P, 20, 2 * D], BF16, isOutput=False)
    wyz_d = nc.declare_dram_parameter("wyzT", [P, 6, 1], BF16, isOutput=False)
    if RZ_DR_STEPS > 0:
        ff8_d = nc.declare_dram_parameter("featF8", [P, 2, GPC * NPAD], FP8,
                                          isOutput=False)
        wih8_d = nc.declare_dram_parameter("WihF8", [P, 2, 2 * D], FP8,
                                           isOutput=False)
        whh8_d = nc.declare_dram_parameter("WhhF8", [P, 2, 2 * D], FP8,
                                           isOutput=False)
    out_d = nc.declare_dram_parameter("out", [1, GPC], FP32, isOutput=True)

    with tile.TileContext(nc) as tc:
        with (
            tc.tile_pool(name="const", bufs=1) as cst,
            tc.tile_pool(name="state", bufs=1) as st,
            tc.tile_pool(name="atp", bufs=3) as atp,
            tc.tile_pool(name="mp", bufs=8) as mp,
            tc.tile_pool(name="rzp", bufs=8) as rzp,
            tc.tile_pool(name="gp", bufs=4) as gp,
            tc.tile_pool(name="cvp", bufs=5) as cvp,
            tc.tile_pool(name="psm", bufs=8, space="PSUM") as psm,
        ):
            # ---- constants ----
            wcat = cst.tile([P, 2, T * D], BF16)
            wih = cst.tile([P, 2, 3 * D], BF16)
            whh = cst.tile([P, 2, 3 * D], BF16)
            biases = cst.tile([P, 22], FP32)
            w12 = cst.tile([P, 8, D], BF16)
            wc = cst.tile([P, 20, 2 * D], BF16)
            wyz = cst.tile([P, 6, 1], BF16)
            if RZ_DR_STEPS > 0:
                wih8 = cst.tile([P, 2, 2 * D], FP8)
                whh8 = cst.tile([P, 2, 2 * D], FP8)
            else:
                wih8 = whh8 = None

            # ---- per-graph state tiles + one-time setup (outside the
            # bench loop: initial h, slot-0 A^T, host-computed binds) ----
            hA, hB, ags, binds, hshs, ags8 = [], [], [], [], [], []
            nc.sync.dma_start(wcat[:], wcat_d[:])
            for g in range(GPC):
                h0 = st.tile([P, 2, NPAD], BF16, tag=f"hA{g}", name=f"hA{g}")
                nc.sync.dma_start(h0[:], hT0_d[:, :, g * NPAD:(g + 1) * NPAD])
                hA.append(h0)
                hB.append(st.tile([P, 2, NPAD], BF16, tag=f"hB{g}", name=f"hB{g}"))
                ags.append(st.tile([P, 2, NPAD], BF16, tag=f"aT{g}", name=f"aT{g}"))
                bnd = st.tile([P, 2, NPAD], BF16, tag=f"bind{g}", name=f"bind{g}")
                nc.sync.dma_start(bnd[:], bind_d[:, :, g * NPAD:(g + 1) * NPAD])
                binds.append(bnd)
                if RZ_DR_STEPS > 0:
                    hsh = st.tile([P, 2, NPAD], FP8, tag=f"hsh{g}", name=f"hsh{g}")
                    nc.sync.dma_start(hsh[:], ff8_d[:, :, g * NPAD:(g + 1) * NPAD])
                    hshs.append(hsh)
                    ags8.append(st.tile([P, 2, NPAD], FP8, tag=f"a8T{g}",
                                        name=f"a8T{g}"))
            at0 = st.tile([P, 16, NPAD], FP8, tag="at0", name="at0")
            nc.sync.dma_start(at0[:], AT_d[0])
            nc.sync.dma_start(wih[:], wih_d[:])
            nc.sync.dma_start(whh[:], whh_d[:])
            if RZ_DR_STEPS > 0:
                nc.sync.dma_start(wih8[:], wih8_d[:])
                nc.sync.dma_start(whh8[:], whh8_d[:])
            nc.sync.dma_start(biases[:], bias_d[:])
            nc.sync.dma_start(w12[:], w12_d[:])
            nc.sync.dma_start(wc[:], wc_d[:])
            nc.sync.dma_start(wyz[:], wyz_d[:])

            # The body is emitted in pairs with swapped hA/hB roles: a body
            # starting from X leaves h0 in Y (its mid-body feat load), which
            # is exactly the swapped body's initial h — no redundant reload,
            # and the loop back-edge seam is paid once per TWO executions.
            hint_engs = tuple(mybir.EngineType[e] for e in
                              ("PE", "DVE", "Activation", "SP", "Pool"))
            pairs, rem = bench_loop // 2, bench_loop % 2
            env = locals()
            if pairs:
                back_edge_label = "bench_backedge"
                with tc.For_i(0, pairs, 1,
                              staggered_reset=True,
                              hint_engines=hint_engs,
                              back_edge_label=back_edge_label) as loop_iv:
                    _kernel_body(nc, tc, env, swap=False, loop_hint=None)
                    # Re-arm the back-edge prefetch near the END of the pair:
                    # the loop-top hint's fetched block is evicted long before
                    # the branch in a body this large (~3.8us I$-stall).
                    _kernel_body(nc, tc, env, swap=True,
                                 loop_hint=(back_edge_label,
                                            loop_iv >= (pairs - 1), hint_engs))
            for _ in range(rem):
                _kernel_body(nc, tc, env, swap=False, loop_hint=None,
                             last=True)
            if not rem and not pairs:
                _kernel_body(nc, tc, env, swap=False, loop_hint=None,
                             last=True)

    nc.compile()
    return nc


def _kernel_body(nc, tc, env, swap=False, loop_hint=None, last=False):
    class E:
        pass
    e = E()
    e.__dict__.update(env)
    e.swap = swap
    e.loop_hint = loop_hint
    e.last = last
    _emit_body(nc, tc, e)


def _emit_body(nc, tc, e):
    (cst, st, atp, mp, rzp, gp, cvp, psm) = (
        e.cst, e.st, e.atp, e.mp, e.rzp, e.gp, e.cvp, e.psm)
    (wcat, wih, whh, biases, w12, wc, wyz) = (
        e.wcat, e.wih, e.whh, e.biases, e.w12, e.wc, e.wyz)
    (hT0_d, out_d, AT_d) = (e.hT0_d, e.out_d, e.AT_d)

    # Per-graph state tiles were created and initialized outside the bench
    # loop (_build_nc): bufX holds this body's initial h (hA on the first
    # body), at0 holds A^T_0, binds hold the b_lin correction. With 6 steps
    # the final h lands back in bufX and the mid-body feat load into bufY
    # doubles as the next (role-swapped) body's initial h.
    (ags, binds, at0) = (e.ags, e.binds, e.at0)
    (hshs, ags8, wih8, whh8) = (e.hshs, e.ags8, e.wih8, e.whh8)
    ff8_d = e.ff8_d if RZ_DR_STEPS > 0 else None
    bufX = e.hB if e.swap else e.hA
    bufY = e.hA if e.swap else e.hB

    # ================= GGNN: software-pipelined slots =================
    def emit_m(s, g):
        """m = h_g @ Wcat (node-major [node, T*D]); also prefetch A^T_g."""
        hg = (bufX if s % 2 == 0 else bufY)[g]
        if s == 0 and g == 0:
            atg = at0
        else:
            atg = atp.tile([P, 16, NPAD], FP8, tag="at")
            nc.sync.dma_start(atg[:], AT_d[g])
        m_tiles = []
        for i in range(4):
            msb = mp.tile([P, T, D], FP8, tag="m")
            for nt in range(2):
                pm = psm.tile([P, 512], FP32, tag="ps")
                for kt in range(2):
                    nc.tensor.matmul(
                        pm[:],
                        lhsT=hg[:, kt, i * P:(i + 1) * P],
                        rhs=wcat[:, kt, nt * 512:(nt + 1) * 512],
                        start=(kt == 0), stop=(kt == 1),
                    )
                dst = msb[:, nt * 2:(nt + 1) * 2, :]
                if (i * 2 + nt) % 8 < 5:
                    nc.vector.tensor_copy(dst, pm[:])
                else:
                    nc.scalar.activation(dst, pm[:], AF.Identity)
            m_tiles.append(msb)
        return m_tiles, atg

    def emit_agg(s, g, m_tiles, atg):
        """aT_g = m_stack^T @ A^T_g + bind_g.

        fp8 DoubleRow: each matmul contracts a (src-tile, type-pair) block
        256 deep (two stacked 128-row k-tiles), with both m and A^T in
        fp8e4 — measured ~2x the per-instruction throughput of the bf16
        pair it replaces (284ns vs 2x296ns at 512 free dim).
        """
        pas = [psm.tile([P, 512], FP32, tag="ps", name=f"pa{mt}")
               for mt in range(2)]
        for c in range(8):
            i, tp = c // 2, c % 2
            for mt in range(2):
                nc.tensor.matmul(
                    pas[mt][:],
                    lhsT=m_tiles[i][:, 2 * tp:2 * tp + 2, mt * P:(mt + 1) * P],
                    rhs=atg[:, 4 * i + 2 * tp:4 * i + 2 * tp + 2, :],
                    start=(c == 0), stop=(c == 7),
                    perf_mode=mybir.MatmulPerfMode.DoubleRow,
                )
        for mt in range(2):
            nc.vector.scalar_tensor_tensor(
                ags[g][:, mt, :], pas[mt][:], 0.0, binds[g][:, mt, :],
                op0=ALU.add, op1=ALU.add)
            if s < RZ_DR_STEPS:
                # fp8 copy of a for the DoubleRow r/z matmuls, on the
                # otherwise-idle Pool engine
                nc.gpsimd.tensor_copy(ags8[g][:, mt, :], ags[g][:, mt, :])

    def emit_gru(s, g):
        hg = (bufX if s % 2 == 0 else bufY)[g]
        hn = (bufY if s % 2 == 0 else bufX)[g]
        agt = ags[g]
        rz_tiles = []
        for mt in range(4):
            pr = psm.tile([P, 512], FP32, tag="ps")
            if s < RZ_DR_STEPS:
                nc.tensor.matmul(
                    pr[:], lhsT=wih8[:, :, mt * P:(mt + 1) * P],
                    rhs=ags8[g][:, :, :], start=True, stop=False,
                    perf_mode=mybir.MatmulPerfMode.DoubleRow)
                nc.tensor.matmul(
                    pr[:], lhsT=whh8[:, :, mt * P:(mt + 1) * P],
                    rhs=hshs[g][:, :, :], start=False, stop=True,
                    perf_mode=mybir.MatmulPerfMode.DoubleRow)
            else:
                for kt in range(2):
                    nc.tensor.matmul(
                        pr[:], lhsT=wih[:, kt, mt * P:(mt + 1) * P],
                        rhs=agt[:, kt, :], start=(kt == 0), stop=False)
                for kt in range(2):
                    nc.tensor.matmul(
                        pr[:], lhsT=whh[:, kt, mt * P:(mt + 1) * P],
                        rhs=hg[:, kt, :], start=False, stop=(kt == 1))
            rzt = rzp.tile([P, 512], BF16, tag="rz")
            nc.scalar.activation(rzt[:], pr[:], AF.Sigmoid,
                                 bias=biases[:, mt:mt + 1])
            rz_tiles.append(rzt)
        for mt in range(2):
            pi = psm.tile([P, 512], FP32, tag="ps")
            for kt in range(2):
                nc.tensor.matmul(
                    pi[:], lhsT=wih[:, kt, 2 * D + mt * P: 2 * D + (mt + 1) * P],
                    rhs=agt[:, kt, :], start=(kt == 0), stop=(kt == 1))
            ph = psm.tile([P, 512], FP32, tag="ps")
            for kt in range(2):
                nc.tensor.matmul(
                    ph[:], lhsT=whh[:, kt, 2 * D + mt * P: 2 * D + (mt + 1) * P],
                    rhs=hg[:, kt, :], start=(kt == 0), stop=(kt == 1))
            t1 = gp.tile([P, 512], BF16, tag="t1")
            nc.vector.scalar_tensor_tensor(
                t1[:], ph[:], biases[:, 6 + mt:7 + mt], rz_tiles[mt][:],
                op0=ALU.add, op1=ALU.mult)
            nc.vector.scalar_tensor_tensor(
                t1[:], pi[:], biases[:, 4 + mt:5 + mt], t1[:],
                op0=ALU.add, op1=ALU.add)
            nsb = gp.tile([P, 512], BF16, tag="nsb")
            nc.scalar.activation(nsb[:], t1[:], AF.Tanh)
            dsb = gp.tile([P, 512], BF16, tag="dsb")
            nc.vector.tensor_tensor(dsb[:], hg[:, mt, :], nsb[:],
                                    op=ALU.subtract)
            nc.vector.tensor_mul(dsb[:], rz_tiles[2 + mt][:], dsb[:])
            nc.vector.tensor_add(hn[:, mt, :], nsb[:], dsb[:])
            if s + 1 < RZ_DR_STEPS:
                # fp8 shadow of h_{s+1} for the next step's DR r/z matmuls
                nc.gpsimd.tensor_copy(hshs[g][:, mt, :], hn[:, mt, :])

    slots = [(s, g) for s in range(NUM_STEPS) for g in range(GPC)]
    nslot = len(slots)
    pend = {}
    for i, (s, g) in enumerate(slots):
        pend[i] = emit_m(s, g)
        if i >= 1:
            s1, g1 = slots[i - 1]
            mt_, at_ = pend.pop(i - 1)
            emit_agg(s1, g1, mt_, at_)
        if i >= 2:
            emit_gru(*slots[i - 2])
    mt_, at_ = pend.pop(nslot - 1)
    emit_agg(*slots[nslot - 1], mt_, at_)
    emit_gru(*slots[nslot - 2])
    emit_gru(*slots[nslot - 1])

    # feat reloads for the Z branch: reuse the bufY buffers (free after the
    # last step read them; NUM_STEPS is even so the final h lives in bufX).
    # This load is also the next (role-swapped) body's initial h.
    for g in range(GPC):
        nc.sync.dma_start(bufY[g][:], hT0_d[:, :, g * NPAD:(g + 1) * NPAD])
        if RZ_DR_STEPS > 0:
            # reset the fp8 h shadow to feat for the next body's step 0
            nc.sync.dma_start(hshs[g][:], ff8_d[:, :, g * NPAD:(g + 1) * NPAD])

    hfin = bufX
    feats = bufY

    # ================= conv readout (pipelined across graphs) =================
    res = cst.tile([1, GPC], FP32)
    y1ps, z1ps, y2ps, z2ps = {}, {}, {}, {}

    def emit_Y1(g):
        hg = hfin[g]
        tiles = []
        for mt in range(2):
            pm = psm.tile([P, 512], FP32, tag="ps")
            first = True
            for k in range(3):
                for kt in range(2):
                    nc.tensor.matmul(
                        pm[:, :L1],
                        lhsT=w12[:, k * 2 + kt, mt * P:(mt + 1) * P],
                        rhs=hg[:, kt, k:k + L1],
                        start=first, stop=(k == 2 and kt == 1))
                    first = False
            y1 = cvp.tile([P, 512], BF16, tag="y1")
            if mt % 2 == 0:
                nc.vector.tensor_scalar(y1[:, :L1], pm[:, :L1],
                                        biases[:, 8 + mt:9 + mt], 0.0,
                                        op0=ALU.add, op1=ALU.max)
            else:
                nc.scalar.activation(y1[:, :L1], pm[:, :L1], AF.Relu,
                                     bias=biases[:, 8 + mt:9 + mt])
            yp = cvp.tile([P, LP1], BF16, tag="y1p")
            nc.vector.tensor_tensor(yp[:], y1[:, 0:505:2], y1[:, 1:506:2],
                                    op=ALU.max)
            nc.vector.tensor_tensor(yp[:], yp[:], y1[:, 2:507:2],
                                    op=ALU.max)
            tiles.append(yp)
        y1ps[g] = tiles

    def emit_Y2(g):
        y1p = y1ps.pop(g)
        tiles = []
        for mt in range(2):
            pm = psm.tile([P, 512], FP32, tag="ps")
            for kt in range(2):
                nc.tensor.matmul(
                    pm[:, :L2Y],
                    lhsT=w12[:, 6 + kt, mt * P:(mt + 1) * P],
                    rhs=y1p[kt][:],
                    start=(kt == 0), stop=(kt == 1))
            y2 = cvp.tile([P, L2Y], BF16, tag="y2")
            if mt % 2 == 0:
                nc.vector.tensor_scalar(y2[:], pm[:, :L2Y],
                                        biases[:, 10 + mt:11 + mt], 0.0,
                                        op0=ALU.add, op1=ALU.max)
            else:
                nc.scalar.activation(y2[:], pm[:, :L2Y], AF.Relu,
                                     bias=biases[:, 10 + mt:11 + mt])
            yp = cvp.tile([P, LF], BF16, tag="y2p")
            nc.vector.tensor_tensor(yp[:], y2[:, 0:251:2], y2[:, 1:252:2],
                                    op=ALU.max)
            tiles.append(yp)
        y2ps[g] = tiles

    def emit_Z1(g):
        hg = hfin[g]
        fg = feats[g]
        tiles = []
        for mt in range(4):
            pm = psm.tile([P, 512], FP32, tag="ps")
            first = True
            for k in range(3):
                for kt in range(4):
                    src = hg if kt < 2 else fg
                    nc.tensor.matmul(
                        pm[:, :L1],
                        lhsT=wc[:, k * 4 + kt, mt * P:(mt + 1) * P],
                        rhs=src[:, kt % 2, k:k + L1],
                        start=first, stop=(k == 2 and kt == 3))
                    first = False
            z1 = cvp.tile([P, 512], BF16, tag="z1")
            if mt % 2 == 0:
                nc.vector.tensor_scalar(z1[:, :L1], pm[:, :L1],
                                        biases[:, 12 + mt:13 + mt], 0.0,
                                        op0=ALU.add, op1=ALU.max)
            else:
                nc.scalar.activation(z1[:, :L1], pm[:, :L1], AF.Relu,
                                     bias=biases[:, 12 + mt:13 + mt])
            zp = cvp.tile([P, LP1], BF16, tag="z1p")
            nc.vector.tensor_tensor(zp[:], z1[:, 0:505:2], z1[:, 1:506:2],
                                    op=ALU.max)
            nc.vector.tensor_tensor(zp[:], zp[:], z1[:, 2:507:2],
                                    op=ALU.max)
            tiles.append(zp)
        z1ps[g] = tiles

    def emit_Z2(g):
        z1p = z1ps.pop(g)
        tiles = []
        for mt in range(4):
            pm = psm.tile([P, 512], FP32, tag="ps")
            first = True
            for k in range(2):
                for kt in range(4):
                    nc.tensor.matmul(
                        pm[:, :L2Z],
                        lhsT=wc[:, 12 + k * 4 + kt, mt * P:(mt + 1) * P],
                        rhs=z1p[kt][:, k:k + L2Z],
                        start=first, stop=(k == 1 and kt == 3))
                    first = False
            z2 = cvp.tile([P, L2Z], BF16, tag="z2")
            if mt % 2 == 0:
                nc.vector.tensor_scalar(z2[:], pm[:, :L2Z],
                                        biases[:, 16 + mt:17 + mt], 0.0,
                                        op0=ALU.add, op1=ALU.max)
            else:
                nc.scalar.activation(z2[:], pm[:, :L2Z], AF.Relu,
                                     bias=biases[:, 16 + mt:17 + mt])
            zp = cvp.tile([P, LF], BF16, tag="z2p")
            nc.vector.tensor_tensor(zp[:], z2[:, 0:251:2], z2[:, 1:252:2],
                                    op=ALU.max)
            tiles.append(zp)
        z2ps[g] = tiles

    def emit_fin(g):
        y2p = y2ps.pop(g)
        z2p = z2ps.pop(g)
        pv = psm.tile([P, 512], FP32, tag="ps")
        for kt in range(2):
            nc.tensor.matmul(pv[0:1, :LF], lhsT=wyz[:, kt, :],
                             rhs=y2p[kt][:], start=(kt == 0), stop=(kt == 1))
        ysb = cvp.tile([1, LF], FP32, tag="ysb")
        nc.scalar.activation(ysb[:], pv[0:1, :LF], AF.Identity,
                             bias=biases[0:1, 20:21])
        pz = psm.tile([P, 512], FP32, tag="ps")
        for kt in range(4):
            nc.tensor.matmul(pz[0:1, :LF], lhsT=wyz[:, 2 + kt, :],
                             rhs=z2p[kt][:], start=(kt == 0), stop=(kt == 3))
        zsb = cvp.tile([1, LF], FP32, tag="zsb")
        nc.scalar.activation(zsb[:], pz[0:1, :LF], AF.Identity,
                             bias=biases[0:1, 21:22])
        prod = cvp.tile([1, LF], FP32, tag="prod")
        nc.vector.tensor_mul(prod[:], ysb[:], zsb[:])
        ssum = cvp.tile([1, 1], FP32, tag="ssum")
        nc.vector.reduce_sum(ssum[:], prod[:], axis=mybir.AxisListType.X)
        nc.scalar.activation(res[:, g:g + 1], ssum[:], AF.Sigmoid,
                             scale=1.0 / LF)

    for g in range(GPC):
        emit_Y1(g)
        if g >= 1:
            emit_Y2(g - 1)
        emit_Z1(g)
        if g >= 1:
            emit_Z2(g - 1)
        if g >= 2:
            emit_fin(g - 2)
    if getattr(e, "loop_hint", None):
        label, hint, engs = e.loop_hint
        tc.mark_branch_hint_location(label, hint=hint, engines=engs)
    emit_Y2(GPC - 1)
    emit_Z2(GPC - 1)
    emit_fin(GPC - 2)
    emit_fin(GPC - 1)
    if not e.last:
        # slot-0 adjacency for the next body (its only reader ran long ago)
        nc.sync.dma_start(at0[:], AT_d[0])

    nc.sync.dma_start(out_d[:], res[:])


def _split_part(a, ntile):
    """[ntile*128, F...] -> [128, ntile, F...] with [p, t, ...] = a[t*128+p, ...]."""
    return np.ascontiguousarray(
        a.reshape(ntile, P, *a.shape[1:]).transpose(1, 0, *range(2, a.ndim + 1)))


def _prep_inputs(inputs):
    feat = np.asarray(inputs["feat"], np.float32)
    esrc = np.asarray(inputs["edge_src"]).astype(np.int64)
    edst = np.asarray(inputs["edge_dst"]).astype(np.int64)
    etyp = np.asarray(inputs["edge_type"]).astype(np.int64)

    # feature-major padded feat^T: per graph [256, 512]
    ftp = np.zeros((B, DIN, NPAD), np.float32)
    ftp[:, :, :N] = feat.transpose(0, 2, 1)

    # adjacency chunks: AT_all[g, c, s, d]; c = (src//128)*4 + t so that the
    # two types of a DoubleRow type-pair are adjacent in the c dim
    AT_all = np.zeros((B, 16, P, NPAD), np.float32)
    DT_all = np.zeros((B, T, NPAD), np.float32)
    g_of = esrc // N
    s_l = esrc % N
    d_l = edst % N
    np.add.at(AT_all, (g_of, (s_l // P) * 4 + etyp, s_l % P, d_l), 1.0)
    np.add.at(DT_all, (g_of, etyp, d_l), 1.0)

    W_lin = np.asarray(inputs["W_lin"], np.float32)
    Wcat = W_lin.transpose(2, 0, 1).reshape(D, T * D)
    b_lin = np.asarray(inputs["b_lin"], np.float32)
    # bind[g, f, dst] = sum_t b_lin[t, f] * indeg_t[g, dst]  (step-invariant
    # b_lin contribution to the aggregation, computed on host)
    bind_all = np.einsum('tf,gtn->gfn', b_lin, DT_all)
    W_ih = np.asarray(inputs["W_ih"], np.float32)
    W_hh = np.asarray(inputs["W_hh"], np.float32)
    b_ih = np.asarray(inputs["b_ih"], np.float32)
    b_hh = np.asarray(inputs["b_hh"], np.float32)

    def convT(w):  # [O, I, K] -> [128, K*ktiles, O]
        O, I, K = w.shape
        kt = I // P
        arr = w.transpose(2, 1, 0).reshape(K, kt, P, O).transpose(2, 0, 1, 3)
        return np.ascontiguousarray(arr.reshape(P, K * kt, O))

    biases = np.zeros((P, 22), np.float32)
    biases[:, 0:4] = (b_ih + b_hh)[:2 * D].reshape(4, P).T
    biases[:, 4:6] = b_ih[2 * D:].reshape(2, P).T
    biases[:, 6:8] = b_hh[2 * D:].reshape(2, P).T
    biases[:, 8:10] = np.asarray(inputs["conv1_b"], np.float32).reshape(2, P).T
    biases[:, 10:12] = np.asarray(inputs["conv2_b"], np.float32).reshape(2, P).T
    biases[:, 12:16] = np.asarray(inputs["convc1_b"], np.float32).reshape(4, P).T
    biases[:, 16:20] = np.asarray(inputs["convc2_b"], np.float32).reshape(4, P).T
    biases[0, 20] = float(np.asarray(inputs["mlp_y_b"])[0])
    biases[0, 21] = float(np.asarray(inputs["mlp_z_b"])[0])
    WihTs = _split_part(np.ascontiguousarray(W_ih.T), 2)
    WhhTs = _split_part(np.ascontiguousarray(W_hh.T), 2)
    common = {
        "Wcat": _split_part(Wcat, 2).astype(bf16),
        "WihT": WihTs.astype(bf16),
        "WhhT": WhhTs.astype(bf16),
        "biases": biases,
        "W12T": np.concatenate(
            [convT(np.asarray(inputs["conv1_w"], np.float32)),
             convT(np.asarray(inputs["conv2_w"], np.float32))], axis=1).astype(bf16),
        "WcT": np.concatenate(
            [convT(np.asarray(inputs["convc1_w"], np.float32)),
             convT(np.asarray(inputs["convc2_w"], np.float32))], axis=1).astype(bf16),
        "wyzT": np.concatenate(
            [_split_part(np.ascontiguousarray(np.asarray(inputs["mlp_y_w"], np.float32).T), 2),
             _split_part(np.ascontiguousarray(np.asarray(inputs["mlp_z_w"], np.float32).T), 4)],
            axis=1).astype(bf16),
    }
    if RZ_DR_STEPS > 0:
        common["WihF8"] = np.ascontiguousarray(
            WihTs[:, :, :2 * D]).astype(fp8)
        common["WhhF8"] = np.ascontiguousarray(
            WhhTs[:, :, :2 * D]).astype(fp8)

    in_maps = []
    for c in range(N_CORES):
        sl = slice(c * GPC, (c + 1) * GPC)
        hT0 = ftp[sl].transpose(1, 0, 2).reshape(DIN, GPC * NPAD)
        m = dict(common)
        hT0s = _split_part(hT0, 2)
        m["hT0"] = hT0s.astype(bf16)
        if RZ_DR_STEPS > 0:
            m["featF8"] = hT0s.astype(fp8)
        m["AT"] = np.ascontiguousarray(
            AT_all[sl].transpose(0, 2, 1, 3)).astype(fp8)
        m["bindT"] = _split_part(
            bind_all[sl].transpose(1, 0, 2).reshape(D, GPC * NPAD), 2
        ).astype(bf16)
        in_maps.append(m)
    return in_maps


def kernel(**inputs):
    if "nc" not in _NC_CACHE:
        _NC_CACHE["nc"] = _build_nc()
    nc = _NC_CACHE["nc"]
    in_maps = _prep_inputs(inputs)
    res = run_bass_kernel_spmd(nc, in_maps, list(range(N_CORES)))
    return np.concatenate([res.results[c]["out"][0] for c in range(N_CORES)])

